# revision 5
# baseline (speedup 1.0000x reference)
"""Trainium2 Bass kernel for masked multi-head attention (B=4, S=1024, D=1024, H=16).

Sharding: 8 cores; core c handles batch b=c//2, query rows [r*512,(r+1)*512) with
r=c%2. No collectives: K/V projection work is duplicated within each core pair
(cheaper than an all-reduce). All matmuls run as float32r (tf32) at full PE rate.

Layouts (per core), everything transposed on the host so contraction dims land on
SBUF partitions:
  xtq [D, 512]  = queries[b, rows].T          xtk/xtv [D, SK] = keys/values[b,:SK].T
  wq, wo [D, D] natural
  vmask [128, NK]: vmask[p,t] = 1.0 if t*128+p < valid_len[b] else 0.0

Pipeline: Q^T = Wq^T-tiles @ xtq; K^T likewise; V natural via lhsT=xtv tiles.
V is stored head-interleaved [sk, 16*(64+1)] with a vmask column per head: the
O^T = V_aug^T @ P^T matmul then yields both the attention output rows (0..63)
and the masked softmax denominator (row 64) in one accumulation. Scores are
computed transposed (S^T[sk, sq] = K_h^T-tile @ Q_h^T), exp(x/8) fused on
ScalarE while copying PSUM->SBUF, masking is purely multiplicative via the
zeroed V rows (exp(NEG)==0 in the reference, identical result).
"""

import numpy as np

import concourse.bass as bass
import concourse.tile as tile
from concourse import bacc, mybir
from concourse.bass_utils import run_bass_kernel_spmd

B, S, D = 4, 1024, 1024
H, HD = 16, 64
N_CORES = 8
SQ = 512  # query rows per core
F32 = mybir.dt.float32
F32R = mybir.dt.float32r
VW = 65  # per-head v_store width (64 dims + 1 mask/ones column)

_module_cache: dict[int, object] = {}


def _build_module(nk: int):
    """Build the SPMD Bass module; nk = number of 128-row key tiles."""
    sk = nk * 128
    nkt = D // 128  # contraction k-tiles for the projections
    nm = D // 128   # output row-tiles (128 douts each)

    nc = bacc.Bacc("TRN2", target_bir_lowering=False, debug=False,
                   num_devices=N_CORES)

    xtq_d = nc.dram_tensor("xtq", [D, SQ], F32, kind="ExternalInput")
    xtk_d = nc.dram_tensor("xtk", [D, sk], F32, kind="ExternalInput")
    xtv_d = nc.dram_tensor("xtv", [D, sk], F32, kind="ExternalInput")
    wq_d = nc.dram_tensor("wq", [D, D], F32, kind="ExternalInput")
    wo_d = nc.dram_tensor("wo", [D, D], F32, kind="ExternalInput")
    vm_d = nc.dram_tensor("vmask", [128, nk], F32, kind="ExternalInput")
    out_d = nc.dram_tensor("outT", [D, SQ], F32, kind="ExternalOutput")

    with tile.TileContext(nc) as tc:
        with (
            tc.tile_pool(name="w", bufs=1) as wpool,
            tc.tile_pool(name="xtq", bufs=1) as xtqpool,
            tc.tile_pool(name="xtkv", bufs=1) as xtkvpool,
            tc.tile_pool(name="qt", bufs=1) as qtpool,
            tc.tile_pool(name="kt", bufs=1) as ktpool,
            tc.tile_pool(name="vs", bufs=1) as vspool,
            tc.tile_pool(name="pt", bufs=2) as ptpool,
            tc.tile_pool(name="ot", bufs=1) as otpool,
            tc.tile_pool(name="small", bufs=1) as smallpool,
            tc.tile_pool(name="inv", bufs=2) as invpool,
            tc.tile_pool(name="dram", bufs=2, space="DRAM") as drampool,
            tc.tile_pool(name="psA", bufs=4, space="PSUM") as psA,
            tc.tile_pool(name="psS", bufs=2, space="PSUM") as psS,
            tc.tile_pool(name="psO", bufs=2, space="PSUM") as psO,
        ):
            # ---- resident weights (wq slot later reused for wo via same tag)
            wq_sb = wpool.tile([128, nkt * D], F32R, tag="w")
            for k in range(nkt):
                nc.sync.dma_start(out=wq_sb[:, k * D:(k + 1) * D],
                                  in_=wq_d.ap()[k * 128:(k + 1) * 128, :].bitcast(F32R))

            vmask_sb = smallpool.tile([128, nk], F32, tag="vmask")
            nc.sync.dma_start(out=vmask_sb[:], in_=vm_d.ap())
            ones16 = smallpool.tile([128, 16], F32, tag="ones16")
            nc.vector.memset(ones16[:], 1.0)

            xtq_sb = xtqpool.tile([128, nkt * SQ], F32R, tag="xtq")
            for k in range(nkt):
                nc.sync.dma_start(out=xtq_sb[:, k * SQ:(k + 1) * SQ],
                                  in_=xtq_d.ap()[k * 128:(k + 1) * 128, :].bitcast(F32R))

            # ---- Q^T projection: qt[dout, sq], row-tile m on partitions
            qt_sb = qtpool.tile([128, nm * SQ], F32R, tag="qt")
            for m in range(nm):
                ps = psA.tile([128, SQ], F32, tag="proj")
                for k in range(nkt):
                    nc.tensor.matmul(
                        ps[:],
                        wq_sb[:, k * D + m * 128: k * D + (m + 1) * 128],
                        xtq_sb[:, k * SQ:(k + 1) * SQ],
                        start=(k == 0), stop=(k == nkt - 1))
                nc.vector.tensor_copy(qt_sb[:, m * SQ:(m + 1) * SQ], ps[:])

            # ---- K^T projection: kt[dout, sk]
            xtk_sb = xtkvpool.tile([128, nkt * sk], F32R, tag="xtkv")
            for k in range(nkt):
                nc.sync.dma_start(out=xtk_sb[:, k * sk:(k + 1) * sk],
                                  in_=xtk_d.ap()[k * 128:(k + 1) * 128, :].bitcast(F32R))
            kt_sb = ktpool.tile([128, nm * sk], F32R, tag="kt")
            nsplits = [(o, min(512, sk - o)) for o in range(0, sk, 512)]
            for m in range(nm):
                for (noff, nw) in nsplits:
                    ps = psA.tile([128, 512], F32, tag="proj")
                    for k in range(nkt):
                        nc.tensor.matmul(
                            ps[:, :nw],
                            wq_sb[:, k * D + m * 128: k * D + (m + 1) * 128],
                            xtk_sb[:, k * sk + noff: k * sk + noff + nw],
                            start=(k == 0), stop=(k == nkt - 1))
                    nc.vector.tensor_copy(
                        kt_sb[:, m * sk + noff: m * sk + noff + nw], ps[:, :nw])

            # ---- V projection into head-interleaved store with mask columns
            xtv_sb = xtkvpool.tile([128, nkt * sk], F32R, tag="xtkv")
            for k in range(nkt):
                nc.sync.dma_start(out=xtv_sb[:, k * sk:(k + 1) * sk],
                                  in_=xtv_d.ap()[k * 128:(k + 1) * 128, :].bitcast(F32R))
            vs_sb = vspool.tile([128, nk * H * VW], F32R, tag="vs")
            for t in range(nk):
                for half in range(2):  # d columns [half*512, half*512+512)
                    ps = psA.tile([128, 512], F32, tag="proj")
                    for k in range(nkt):
                        nc.tensor.matmul(
                            ps[:],
                            xtv_sb[:, k * sk + t * 128: k * sk + (t + 1) * 128],
                            wq_sb[:, k * D + half * 512: k * D + half * 512 + 512],
                            start=(k == 0), stop=(k == nkt - 1))
                    dst = vs_sb[:, t * H * VW + half * 8 * VW:
                                t * H * VW + (half + 1) * 8 * VW]
                    dst = dst.rearrange("p (h c) -> p h c", c=VW)[:, :, 0:HD]
                    src = ps[:].rearrange("p (h c) -> p h c", c=HD)
                    nc.vector.tensor_scalar_mul(dst, src, vmask_sb[:, t:t + 1])
                mcols = vs_sb[:, t * H * VW: (t + 1) * H * VW]
                mcols = mcols.rearrange("p (h c) -> p h c", c=VW)[:, :, HD:VW]
                nc.vector.tensor_scalar_mul(
                    mcols, ones16[:].rearrange("p (h o) -> p h o", o=1),
                    vmask_sb[:, t:t + 1])

            # wo loads into the wq slot; Tile serializes on wq's last reader
            wo_sb = wpool.tile([128, nkt * D], F32R, tag="w")
            for k in range(nkt):
                nc.sync.dma_start(out=wo_sb[:, k * D:(k + 1) * D],
                                  in_=wo_d.ap()[k * 128:(k + 1) * 128, :].bitcast(F32R))

            # ---- attention per head
            ot_sb = otpool.tile([128, nm * SQ], F32R, tag="ot")
            for h in range(H):
                po = 64 * (h % 2)       # partition offset of this head's douts
                mb = h // 2             # dout row-tile holding this head
                pt = ptpool.tile([128, nk * SQ], F32R, tag="pt")
                for t in range(nk):
                    ss = psS.tile([128, SQ], F32, tag="s")
                    nc.tensor.matmul(
                        ss[:],
                        kt_sb[po:po + 64, mb * sk + t * 128: mb * sk + (t + 1) * 128],
                        qt_sb[po:po + 64, mb * SQ:(mb + 1) * SQ],
                        start=True, stop=True)
                    nc.scalar.activation(pt[:, t * SQ:(t + 1) * SQ], ss[:],
                                         mybir.ActivationFunctionType.Exp,
                                         scale=0.125)
                po_ps = psO.tile([VW, SQ], F32, tag="o")
                for t in range(nk):
                    nc.tensor.matmul(
                        po_ps[:],
                        vs_sb[:, t * H * VW + h * VW: t * H * VW + (h + 1) * VW],
                        pt[:, t * SQ:(t + 1) * SQ],
                        start=(t == 0), stop=(t == nk - 1))
                inv = invpool.tile([1, SQ], F32, tag="inv")
                nc.vector.reciprocal(inv[:], po_ps[64:65, :])
                inv_dr = drampool.tile([1, SQ], F32, tag="invdr")
                nc.sync.dma_start(out=inv_dr[:], in_=inv[:])
                inv_rep = invpool.tile([64, SQ], F32, tag="invrep")
                nc.sync.dma_start(out=inv_rep[:],
                                  in_=inv_dr[0:1, :].partition_broadcast(64))
                nc.vector.tensor_mul(
                    ot_sb[po:po + 64, mb * SQ:(mb + 1) * SQ],
                    po_ps[0:64, :], inv_rep[:])

            # ---- output projection: outT[dout, sq] = Wo^T-tiles @ O^T
            for m in range(nm):
                ps = psA.tile([128, SQ], F32, tag="proj")
                for k in range(nkt):
                    nc.tensor.matmul(
                        ps[:],
                        wo_sb[:, k * D + m * 128: k * D + (m + 1) * 128],
                        ot_sb[:, k * SQ:(k + 1) * SQ],
                        start=(k == 0), stop=(k == nkt - 1))
                osb = invpool.tile([128, SQ], F32, tag="outsb")
                nc.vector.tensor_copy(osb[:], ps[:])
                nc.sync.dma_start(out=out_d.ap()[m * 128:(m + 1) * 128, :],
                                  in_=osb[:])

    nc.compile()
    return nc


def kernel(queries, keys, values, valid_lengths, W_q, W_o):
    queries = np.ascontiguousarray(np.asarray(queries, dtype=np.float32))
    keys = np.ascontiguousarray(np.asarray(keys, dtype=np.float32))
    values = np.ascontiguousarray(np.asarray(values, dtype=np.float32))
    W_q = np.ascontiguousarray(np.asarray(W_q, dtype=np.float32))
    W_o = np.ascontiguousarray(np.asarray(W_o, dtype=np.float32))
    vls = np.asarray(valid_lengths).astype(np.int64)

    nk = max(1, int(-(-int(vls.max()) // 128)))  # ceil(max_vl/128)
    sk = nk * 128

    nc = _module_cache.get(nk)
    if nc is None:
        nc = _build_module(nk)
        _module_cache[nk] = nc

    in_maps = []
    for c in range(N_CORES):
        b, r = c // 2, c % 2
        vl = int(vls[b])
        vm = (np.arange(sk) < vl).astype(np.float32).reshape(nk, 128).T
        in_maps.append({
            "xtq": np.ascontiguousarray(queries[b, r * SQ:(r + 1) * SQ, :].T),
            "xtk": np.ascontiguousarray(keys[b, :sk, :].T),
            "xtv": np.ascontiguousarray(values[b, :sk, :].T),
            "wq": W_q,
            "wo": W_o,
            "vmask": np.ascontiguousarray(vm),
        })

    res = run_bass_kernel_spmd(nc, in_maps, list(range(N_CORES)))

    out = np.empty((B, S, D), dtype=np.float32)
    for c in range(N_CORES):
        b, r = c // 2, c % 2
        out[b, r * SQ:(r + 1) * SQ, :] = res.results[c]["outT"].T
    return out
